# Initial kernel scaffold
#
"""Trainium2 Bass kernel for nn_ComplexConv2Deffangle4Dxy.

Reference math (per batch b, branch br):
    out[br] = pointwise(w2, depthwise3x3(w1, img[br]))   with zero padding P=1
      br=0 (rot): weights (w1n, w2n) where wn = (wx+wy)^2 / sum((wx+wy)^2)
      br=1 (abs): log-domain: exp(branch(log(img + EPS), w1n, w2n))
      br=2 (x):   weights (w1x, w2x)
      br=3 (y):   weights (w1y, w2y)

Kernel strategy (per NeuronCore, data-parallel over batch B=8 -> 8 cores):
  Fuse depthwise+pointwise into a single 3x3 conv whose weights are the
  outer product  Wf[o, c, k] = w2[o, c] * w1[c, k].  The conv is computed
  as 9 PSUM-accumulated matmuls (one per kernel offset k) with
  lhsT = Wf[:, :, k]  (K=Cin=64, M=Cout=128)  and
  rhs  = shifted image view (K=64, N=512).
  The H dimension is split across the two partition halves (rows 0..31 of
  the output come from SBUF partitions 0..63, rows 32..63 from partitions
  64..127) so each offset issues two K=64 matmuls on disjoint PE row
  groups which execute concurrently (row tiling).  Images are zero-padded
  on the host (pure marshaling) so every shifted view is a plain strided
  AP with no boundary special cases; for the abs branch Ln(x*1+EPS) maps
  the zero padding to log(EPS), exactly matching the reference's
  pad-then-log order.  Weight normalization for the rot/abs branches is
  computed on device (sum via ones-matmul, reciprocal on DVE, scale folded
  into the fused conv weights).
"""

import sys

for _p in ("/opt/trn_rl_repo",):
    if _p not in sys.path:
        sys.path.insert(0, _p)

import numpy as np

import concourse.bacc as bacc
import concourse.mybir as mybir
import concourse.tile as tile
from concourse import bass_utils

F32 = mybir.dt.float32
F32R = mybir.dt.float32r

EPS = 1e-6
N_CORES = 8
B, NBR, CIN, COUT, H, W = 8, 4, 64, 128, 64, 64
HP, WP = H + 2, W + 2          # host-padded image
HALF_ROWS = 35                 # padded rows per partition half (with halo)
ROW_TILES = 4                  # 4 output-row tiles of 8 rows per half
TILE_N = 512                   # 8 output rows x 64 cols = one PSUM bank fp32

# matmul input dtype: "f32r" (full-speed fp32-relaxed) or "f32" (4x slower, exact)
MM_DTYPE = "f32r"
TRACE = False
LAST_EXEC_TIME_NS = None
LAST_RESULTS = None

_PROG_CACHE = {}


def _mm_cast(ap):
    if MM_DTYPE == "f32r":
        return ap.bitcast(F32R)
    return ap


def _emit(nc, tc, xin_d, w1x_d, w1y_d, w2xT_d, w2yT_d, out_d):
    wp = tc.tile_pool(name="wp", bufs=1)
    imgp = tc.tile_pool(name="imgp", bufs=2)
    psr = tc.tile_pool(name="psr", bufs=2, space="PSUM")
    psp = tc.tile_pool(name="psp", bufs=6, space="PSUM")
    obp = tc.tile_pool(name="obp", bufs=6)
    with wp, imgp, psr, psp, obp:
        # ---- weight prep -------------------------------------------------
        w1x_s = wp.tile([CIN, 9], F32, tag="w1x")
        w1y_s = wp.tile([CIN, 9], F32, tag="w1y")
        w2xT_s = wp.tile([CIN, COUT], F32, tag="w2xT")
        w2yT_s = wp.tile([CIN, COUT], F32, tag="w2yT")
        nc.sync.dma_start(out=w1x_s[:, :], in_=w1x_d)
        nc.sync.dma_start(out=w1y_s[:, :], in_=w1y_d)
        nc.sync.dma_start(out=w2xT_s[:, :], in_=w2xT_d)
        nc.sync.dma_start(out=w2yT_s[:, :], in_=w2yT_d)

        ones_k = wp.tile([CIN, 1], F32, tag="ones_k")
        nc.vector.memset(ones_k[:, :], 1.0)
        ones_m = wp.tile([1, CIN], F32, tag="ones_m")
        nc.vector.memset(ones_m[:, :], 1.0)

        # u1 = (w1x + w1y)^2   (64, 9)
        u1 = wp.tile([CIN, 9], F32, tag="u1")
        nc.vector.tensor_add(u1[:, :], w1x_s[:, :], w1y_s[:, :])
        nc.vector.tensor_mul(u1[:, :], u1[:, :], u1[:, :])
        # u2T = ((w2x + w2y)^2)^T  (64, 128)
        u2T = wp.tile([CIN, COUT], F32, tag="u2T")
        nc.vector.tensor_add(u2T[:, :], w2xT_s[:, :], w2yT_s[:, :])
        nc.vector.tensor_mul(u2T[:, :], u2T[:, :], u2T[:, :])

        # S1 = sum(u1), S2 = sum(u2) via ones-matmul + free-dim reduce
        s1v = psr.tile([1, 9], F32, tag="red")
        nc.tensor.matmul(s1v[:, :], ones_k[:, :], u1[:, :], start=True, stop=True)
        s2v = psr.tile([1, COUT], F32, tag="red")
        nc.tensor.matmul(s2v[:, :], ones_k[:, :], u2T[:, :], start=True, stop=True)
        s1 = wp.tile([1, 1], F32, tag="s1")
        nc.vector.tensor_reduce(
            s1[:, :], s1v[:, :], axis=mybir.AxisListType.X, op=mybir.AluOpType.add
        )
        s2 = wp.tile([1, 1], F32, tag="s2")
        nc.vector.tensor_reduce(
            s2[:, :], s2v[:, :], axis=mybir.AxisListType.X, op=mybir.AluOpType.add
        )
        inv = wp.tile([1, 1], F32, tag="inv")
        nc.vector.tensor_mul(inv[:, :], s1[:, :], s2[:, :])
        nc.vector.reciprocal(inv[:, :], inv[:, :])
        # broadcast 1/(S1*S2) to 64 partitions
        invb_ps = psr.tile([CIN, 1], F32, tag="red")
        nc.tensor.matmul(invb_ps[:, :], ones_m[:, :], inv[:, :], start=True, stop=True)
        invb = wp.tile([CIN, 1], F32, tag="invb")
        nc.vector.tensor_copy(invb[:, :], invb_ps[:, :])
        # u2T_n = u2T * 1/(S1*S2): both normalizations folded into one scale
        u2Tn = wp.tile([CIN, COUT], F32, tag="u2Tn")
        nc.vector.tensor_scalar(
            u2Tn[:, :], u2T[:, :], invb[:, 0:1], None, mybir.AluOpType.mult
        )

        # fused conv weights per set: Wf[c, k*128 + o] = w2T[c, o] * w1[c, k]
        # (replicated into partitions 64..127 for the upper-half row group)
        wf_tiles = {}
        for s, base, w1s in (("x", w2xT_s, w1x_s), ("y", w2yT_s, w1y_s), ("n", u2Tn, u1)):
            wf = wp.tile([2 * CIN, 9 * COUT], F32, tag=f"wf{s}")
            for k in range(9):
                nc.vector.tensor_scalar(
                    wf[0:CIN, k * COUT : (k + 1) * COUT],
                    base[:, :],
                    w1s[:, k : k + 1],
                    None,
                    mybir.AluOpType.mult,
                )
            nc.sync.dma_start(out=wf[CIN : 2 * CIN, :], in_=wf[0:CIN, :])
            wf_tiles[s] = wf

        # ---- main loop ---------------------------------------------------
        # branch order: x, y first (their weights are ready earliest)
        for b, s, needs_log, evac in (
            (2, "x", False, "v"),
            (3, "y", False, "a"),
            (0, "n", False, "v"),
            (1, "n", True, "a"),
        ):
            wf = wf_tiles[s]
            img = imgp.tile([2 * CIN, HALF_ROWS, WP], F32, tag="img")
            nc.sync.dma_start(out=img[0:CIN], in_=xin_d[b, 0])
            nc.sync.dma_start(out=img[CIN : 2 * CIN], in_=xin_d[b, 1])
            if needs_log:
                # log(x + EPS); padding zeros become log(EPS) as in reference
                nc.scalar.activation(
                    img[:, :, :],
                    img[:, :, :],
                    mybir.ActivationFunctionType.Ln,
                    bias=float(EPS),
                )
            for tp in range(ROW_TILES):
                ps_lo = psp.tile([COUT, 8, W], F32, tag="ps")
                ps_hi = psp.tile([COUT, 8, W], F32, tag="ps")
                for k in range(9):
                    dh, dw = k // 3 - 1, k % 3 - 1
                    r_lo = 8 * tp + 1 + dh        # lower half: padded row - 0
                    r_hi = 8 * tp + 2 + dh        # upper half: padded row - 31
                    c0 = 1 + dw
                    nc.tensor.matmul(
                        ps_lo[:, :, :],
                        _mm_cast(wf[0:CIN, k * COUT : (k + 1) * COUT]),
                        _mm_cast(img[0:CIN, r_lo : r_lo + 8, c0 : c0 + W]),
                        start=(k == 0),
                        stop=(k == 8),
                    )
                    nc.tensor.matmul(
                        ps_hi[:, :, :],
                        _mm_cast(wf[CIN : 2 * CIN, k * COUT : (k + 1) * COUT]),
                        _mm_cast(img[CIN : 2 * CIN, r_hi : r_hi + 8, c0 : c0 + W]),
                        start=(k == 0),
                        stop=(k == 8),
                    )
                for half, ps in ((0, ps_lo), (1, ps_hi)):
                    h0 = 32 * half + 8 * tp
                    ot = obp.tile([COUT, 8, W], F32, tag="ot")
                    if needs_log:
                        nc.scalar.activation(
                            ot[:, :, :], ps[:, :, :], mybir.ActivationFunctionType.Exp
                        )
                    elif evac == "v":
                        nc.vector.tensor_copy(ot[:, :, :], ps[:, :, :])
                    else:
                        nc.scalar.activation(
                            ot[:, :, :], ps[:, :, :], mybir.ActivationFunctionType.Copy
                        )
                    nc.sync.dma_start(out=out_d[b, :, h0 : h0 + 8, :], in_=ot[:, :, :])


def build_program():
    key = MM_DTYPE
    if key in _PROG_CACHE:
        return _PROG_CACHE[key]
    nc = bacc.Bacc("TRN2", target_bir_lowering=False, debug=False)
    xin_d = nc.dram_tensor("xin", [NBR, 2, CIN, HALF_ROWS, WP], F32, kind="ExternalInput").ap()
    w1x_d = nc.dram_tensor("w1x", [CIN, 9], F32, kind="ExternalInput").ap()
    w1y_d = nc.dram_tensor("w1y", [CIN, 9], F32, kind="ExternalInput").ap()
    w2xT_d = nc.dram_tensor("w2xT", [CIN, COUT], F32, kind="ExternalInput").ap()
    w2yT_d = nc.dram_tensor("w2yT", [CIN, COUT], F32, kind="ExternalInput").ap()
    out_d = nc.dram_tensor("out", [NBR, COUT, H, W], F32, kind="ExternalOutput").ap()
    with tile.TileContext(nc) as tc:
        _emit(nc, tc, xin_d, w1x_d, w1y_d, w2xT_d, w2yT_d, out_d)
    nc.compile()
    _PROG_CACHE[key] = nc
    return nc


def marshal_inputs(x, w1x, w1y, w2x, w2y):
    """Host-side data marshaling: shard over batch, zero-pad, split H halves."""
    x = np.ascontiguousarray(x, dtype=np.float32)
    xp = np.zeros((B, NBR, CIN, HP, WP), np.float32)
    xp[:, :, :, 1 : H + 1, 1 : W + 1] = x
    xin = np.empty((B, NBR, 2, CIN, HALF_ROWS, WP), np.float32)
    xin[:, :, 0] = xp[:, :, :, 0:HALF_ROWS, :]          # padded rows 0..34
    xin[:, :, 1] = xp[:, :, :, HP - HALF_ROWS : HP, :]  # padded rows 31..65
    w2xT = np.ascontiguousarray(np.asarray(w2x, np.float32).T)
    w2yT = np.ascontiguousarray(np.asarray(w2y, np.float32).T)
    w1x = np.ascontiguousarray(w1x, np.float32)
    w1y = np.ascontiguousarray(w1y, np.float32)
    return [
        {
            "xin": np.ascontiguousarray(xin[i]),
            "w1x": w1x,
            "w1y": w1y,
            "w2xT": w2xT,
            "w2yT": w2yT,
        }
        for i in range(B)
    ]


def kernel(x, w1x, w1y, w2x, w2y):
    global LAST_EXEC_TIME_NS, LAST_RESULTS
    nc = build_program()
    in_maps = marshal_inputs(x, w1x, w1y, w2x, w2y)
    res = bass_utils.run_bass_kernel_spmd(
        nc, in_maps, list(range(N_CORES)), trace=TRACE
    )
    LAST_EXEC_TIME_NS = res.exec_time_ns
    LAST_RESULTS = res
    out = np.stack([res.results[i]["out"] for i in range(N_CORES)], axis=0)
    return out.astype(np.float32, copy=False)


# revision 9
# speedup vs baseline: 1.5812x; 1.5812x over previous
"""Trainium2 Bass kernel for nn_ComplexConv2Deffangle4Dxy.

Reference math (per batch b, branch br):
    out[br] = pointwise(w2, depthwise3x3(w1, img[br]))   with zero padding P=1
      br=0 (rot): weights (w1n, w2n) where wn = (wx+wy)^2 / sum((wx+wy)^2)
      br=1 (abs): log-domain: exp(branch(log(img + EPS), w1n, w2n))
      br=2 (x):   weights (w1x, w2x)
      br=3 (y):   weights (w1y, w2y)

Kernel strategy (per NeuronCore, data-parallel over batch B=8 -> 8 cores):
  Fuse depthwise+pointwise into a single 3x3 conv whose weights are the
  outer product  Wf[o, c, k] = w2[o, c] * w1[c, k].  The conv is computed
  as 9 PSUM-accumulated matmuls (one per kernel offset k) with
  lhsT = Wf[:, :, k]  (K=Cin=64, M=Cout=128)  and
  rhs  = shifted image view (K=64, N=512).
  The H dimension is split across the two partition halves (rows 0..31 of
  the output come from SBUF partitions 0..63, rows 32..63 from partitions
  64..127) so each offset issues two K=64 matmuls on disjoint PE row
  groups which execute concurrently (row tiling).  Images are zero-padded
  on the host (pure marshaling) so every shifted view is a plain strided
  AP with no boundary special cases; for the abs branch Ln(x*1+EPS) maps
  the zero padding to log(EPS), exactly matching the reference's
  pad-then-log order.  Weight normalization for the rot/abs branches is
  computed on device (sum via ones-matmul, reciprocal on DVE, scale folded
  into the fused conv weights).
"""

import sys

for _p in ("/opt/trn_rl_repo",):
    if _p not in sys.path:
        sys.path.insert(0, _p)

import numpy as np

import concourse.bacc as bacc
import concourse.mybir as mybir
import concourse.tile as tile
from concourse import bass_utils

F32 = mybir.dt.float32
F32R = mybir.dt.float32r

EPS = 1e-6
N_CORES = 8
B, NBR, CIN, COUT, H, W = 8, 4, 64, 128, 64, 64
HP, WP = H + 2, W + 2          # host-padded image
HALF_ROWS = 35                 # padded rows per partition half (with halo)
ROW_TILES = 4                  # 4 output-row tiles of 8 rows per half
TILE_N = 512                   # 8 output rows x 64 cols = one PSUM bank fp32

# matmul input dtype: "f32r" (full-speed fp32-relaxed) or "f32" (4x slower, exact)
MM_DTYPE = "f32r"
# when set (benchmarking only), wraps the main compute in a device-side
# repeat loop so per-iteration time is resolvable from host wall clock
LOOP_ITERS = None
TRACE = False
LAST_EXEC_TIME_NS = None
LAST_RESULTS = None

_PROG_CACHE = {}


def _mm_dt():
    return F32R if MM_DTYPE == "f32r" else F32


def _emit(nc, tc, xin_d, w1x_d, w1y_d, w2xT_d, w2yT_d, out_d):
    with (
        tc.tile_pool(name="wp", bufs=1) as wp,
        tc.tile_pool(name="imgp", bufs=2) as imgp,
        tc.tile_pool(name="psr", bufs=2, space="PSUM") as psr,
        tc.tile_pool(name="psp", bufs=6, space="PSUM") as psp,
        tc.tile_pool(name="obp", bufs=6) as obp,
    ):
        # ---- weight prep -------------------------------------------------
        w1x_s = wp.tile([CIN, 9], F32, tag="w1x")
        w1y_s = wp.tile([CIN, 9], F32, tag="w1y")
        w2xT_s = wp.tile([CIN, COUT], F32, tag="w2xT")
        w2yT_s = wp.tile([CIN, COUT], F32, tag="w2yT")
        nc.sync.dma_start(out=w1x_s[:, :], in_=w1x_d)
        nc.sync.dma_start(out=w1y_s[:, :], in_=w1y_d)
        nc.sync.dma_start(out=w2xT_s[:, :], in_=w2xT_d)
        nc.sync.dma_start(out=w2yT_s[:, :], in_=w2yT_d)

        ones_k = wp.tile([CIN, 1], F32, tag="ones_k")
        nc.vector.memset(ones_k[:, :], 1.0)
        ones_m = wp.tile([1, CIN], F32, tag="ones_m")
        nc.vector.memset(ones_m[:, :], 1.0)
        eps_b = wp.tile([2 * CIN, 1], F32, tag="eps_b")
        nc.vector.memset(eps_b[:, :], float(EPS))
        zero_b = wp.tile([COUT, 1], F32, tag="zero_b")
        nc.vector.memset(zero_b[:, :], 0.0)

        # u1 = (w1x + w1y)^2   (64, 9)
        u1 = wp.tile([CIN, 9], F32, tag="u1")
        nc.vector.tensor_add(u1[:, :], w1x_s[:, :], w1y_s[:, :])
        nc.vector.tensor_mul(u1[:, :], u1[:, :], u1[:, :])
        # u2T = ((w2x + w2y)^2)^T  (64, 128)
        u2T = wp.tile([CIN, COUT], F32, tag="u2T")
        nc.vector.tensor_add(u2T[:, :], w2xT_s[:, :], w2yT_s[:, :])
        nc.vector.tensor_mul(u2T[:, :], u2T[:, :], u2T[:, :])

        # S1 = sum(u1), S2 = sum(u2) via ones-matmul + free-dim reduce
        s1v = psr.tile([1, 9], F32, tag="red")
        nc.tensor.matmul(s1v[:, :], ones_k[:, :], u1[:, :], start=True, stop=True)
        s2v = psr.tile([1, COUT], F32, tag="red")
        nc.tensor.matmul(s2v[:, :], ones_k[:, :], u2T[:, :], start=True, stop=True)
        s1 = wp.tile([1, 1], F32, tag="s1")
        nc.vector.tensor_reduce(
            s1[:, :], s1v[:, :], axis=mybir.AxisListType.X, op=mybir.AluOpType.add
        )
        s2 = wp.tile([1, 1], F32, tag="s2")
        nc.vector.tensor_reduce(
            s2[:, :], s2v[:, :], axis=mybir.AxisListType.X, op=mybir.AluOpType.add
        )
        inv = wp.tile([1, 1], F32, tag="inv")
        nc.vector.tensor_mul(inv[:, :], s1[:, :], s2[:, :])
        nc.vector.reciprocal(inv[:, :], inv[:, :])
        # broadcast 1/(S1*S2) to 64 partitions
        invb_ps = psr.tile([CIN, 1], F32, tag="red")
        nc.tensor.matmul(invb_ps[:, :], ones_m[:, :], inv[:, :], start=True, stop=True)
        invb = wp.tile([CIN, 1], F32, tag="invb")
        nc.vector.tensor_copy(invb[:, :], invb_ps[:, :])
        # u2T_n = u2T * 1/(S1*S2): both normalizations folded into one scale
        u2Tn = wp.tile([CIN, COUT], F32, tag="u2Tn")
        nc.vector.tensor_scalar(
            u2Tn[:, :], u2T[:, :], invb[:, 0:1], None, mybir.AluOpType.mult
        )

        # fused conv weights per set: Wf[c, k*128 + o] = w2T[c, o] * w1[c, k]
        # (replicated into partitions 64..127 for the upper-half row group)
        wf_tiles = {}
        for s, base, w1s in (("x", w2xT_s, w1x_s), ("y", w2yT_s, w1y_s), ("n", u2Tn, u1)):
            wf = wp.tile([2 * CIN, 9 * COUT], _mm_dt(), tag=f"wf{s}")
            for k in range(9):
                nc.vector.tensor_scalar(
                    wf[0:CIN, k * COUT : (k + 1) * COUT],
                    base[:, :],
                    w1s[:, k : k + 1],
                    None,
                    mybir.AluOpType.mult,
                )
            nc.sync.dma_start(out=wf[CIN : 2 * CIN, :], in_=wf[0:CIN, :])
            wf_tiles[s] = wf

        # ---- main loop ---------------------------------------------------
        def main_body():
            _emit_main(nc, tc, imgp, psp, obp, wf_tiles, eps_b, zero_b, xin_d, out_d)

        if LOOP_ITERS:
            with tc.For_i(0, LOOP_ITERS, 1):
                main_body()
        else:
            main_body()


def _emit_main(nc, tc, imgp, psp, obp, wf_tiles, eps_b, zero_b, xin_d, out_d):
        # branch order: x, y first (their weights are ready earliest)
        for b, s, needs_log, evac in (
            (2, "x", False, "v"),
            (3, "y", False, "a"),
            (0, "n", False, "v"),
            (1, "n", True, "a"),
        ):
            wf = wf_tiles[s]
            img = imgp.tile([2 * CIN, HALF_ROWS, WP], _mm_dt(), tag="img")
            nc.sync.dma_start(out=img[0:CIN], in_=xin_d[b, 0])
            nc.sync.dma_start(out=img[CIN : 2 * CIN], in_=xin_d[b, 1])
            if needs_log:
                # log(x + EPS); padding zeros become log(EPS) as in reference
                nc.scalar.activation(
                    img[:, :, :],
                    img[:, :, :],
                    mybir.ActivationFunctionType.Ln,
                    bias=eps_b[:, 0:1],
                )
            for tp in range(ROW_TILES):
                ps_lo = psp.tile([COUT, 8, W], F32, tag="ps")
                ps_hi = psp.tile([COUT, 8, W], F32, tag="ps")
                for k in range(9):
                    dh, dw = k // 3 - 1, k % 3 - 1
                    r_lo = 8 * tp + 1 + dh        # lower half: padded row - 0
                    r_hi = 8 * tp + 2 + dh        # upper half: padded row - 31
                    c0 = 1 + dw
                    nc.tensor.matmul(
                        ps_lo[:, :, :],
                        wf[0:CIN, k * COUT : (k + 1) * COUT],
                        img[0:CIN, r_lo : r_lo + 8, c0 : c0 + W],
                        start=(k == 0),
                        stop=(k == 8),
                    )
                    nc.tensor.matmul(
                        ps_hi[:, :, :],
                        wf[CIN : 2 * CIN, k * COUT : (k + 1) * COUT],
                        img[CIN : 2 * CIN, r_hi : r_hi + 8, c0 : c0 + W],
                        start=(k == 0),
                        stop=(k == 8),
                    )
                for half, ps in ((0, ps_lo), (1, ps_hi)):
                    h0 = 32 * half + 8 * tp
                    ot = obp.tile([COUT, 8, W], F32, tag="ot")
                    if needs_log:
                        nc.scalar.activation(
                            ot[:, :, :],
                            ps[:, :, :],
                            mybir.ActivationFunctionType.Exp,
                            bias=zero_b[:, 0:1],
                        )
                    elif evac == "v":
                        nc.vector.tensor_copy(ot[:, :, :], ps[:, :, :])
                    else:
                        nc.scalar.activation(
                            ot[:, :, :], ps[:, :, :], mybir.ActivationFunctionType.Copy
                        )
                    nc.sync.dma_start(out=out_d[b, :, h0 : h0 + 8, :], in_=ot[:, :, :])


def build_program():
    key = (MM_DTYPE, LOOP_ITERS)
    if key in _PROG_CACHE:
        return _PROG_CACHE[key]
    nc = bacc.Bacc("TRN2", target_bir_lowering=False, debug=False)
    xin_d = nc.dram_tensor("xin", [NBR, 2, CIN, HALF_ROWS, WP], _mm_dt(), kind="ExternalInput").ap()
    w1x_d = nc.dram_tensor("w1x", [CIN, 9], F32, kind="ExternalInput").ap()
    w1y_d = nc.dram_tensor("w1y", [CIN, 9], F32, kind="ExternalInput").ap()
    w2xT_d = nc.dram_tensor("w2xT", [CIN, COUT], F32, kind="ExternalInput").ap()
    w2yT_d = nc.dram_tensor("w2yT", [CIN, COUT], F32, kind="ExternalInput").ap()
    out_d = nc.dram_tensor("out", [NBR, COUT, H, W], F32, kind="ExternalOutput").ap()
    with tile.TileContext(nc) as tc:
        _emit(nc, tc, xin_d, w1x_d, w1y_d, w2xT_d, w2yT_d, out_d)
    nc.compile()
    _PROG_CACHE[key] = nc
    return nc


def marshal_inputs(x, w1x, w1y, w2x, w2y):
    """Host-side data marshaling: shard over batch, zero-pad, split H halves."""
    x = np.ascontiguousarray(x, dtype=np.float32)
    xp = np.zeros((B, NBR, CIN, HP, WP), np.float32)
    xp[:, :, :, 1 : H + 1, 1 : W + 1] = x
    xin = np.empty((B, NBR, 2, CIN, HALF_ROWS, WP), np.float32)
    xin[:, :, 0] = xp[:, :, :, 0:HALF_ROWS, :]          # padded rows 0..34
    xin[:, :, 1] = xp[:, :, :, HP - HALF_ROWS : HP, :]  # padded rows 31..65
    w2xT = np.ascontiguousarray(np.asarray(w2x, np.float32).T)
    w2yT = np.ascontiguousarray(np.asarray(w2y, np.float32).T)
    w1x = np.ascontiguousarray(w1x, np.float32)
    w1y = np.ascontiguousarray(w1y, np.float32)
    return [
        {
            "xin": np.ascontiguousarray(xin[i]),
            "w1x": w1x,
            "w1y": w1y,
            "w2xT": w2xT,
            "w2yT": w2yT,
        }
        for i in range(B)
    ]


def kernel(x, w1x, w1y, w2x, w2y):
    global LAST_EXEC_TIME_NS, LAST_RESULTS
    nc = build_program()
    in_maps = marshal_inputs(x, w1x, w1y, w2x, w2y)
    res = bass_utils.run_bass_kernel_spmd(
        nc, in_maps, list(range(N_CORES)), trace=TRACE
    )
    LAST_EXEC_TIME_NS = res.exec_time_ns
    LAST_RESULTS = res
    out = np.stack([res.results[i]["out"] for i in range(N_CORES)], axis=0)
    return out.astype(np.float32, copy=False)
